# revision 61
# baseline (speedup 1.0000x reference)
"""Self-contained Trainium2 Bass kernel: pre-LN multi-head attention block.

Computes, for x [B=8, S=1024, D=1024] (fp32) and packed attention weights:
    out = x + out_proj(MHA(LayerNorm(x)))
matching torch nn.MultiheadAttention's explicit (non-flash) path with 16 heads.

Sharding: data-parallel over batch - core i handles batch element i; no
collectives, outputs are concatenated on the host.

v3 strategy (fp8 DoubleRow matmuls, scalar-engine-bound softmax):
  - LN gamma/beta are folded into the projection weights/biases host-side;
    the V-projection bias passes through softmax additively (sum p = 1) so it
    folds into the residual too.  Device LN is z = (x - mu) * rstd with stats
    via all-ones matmuls (d on partitions); z is written directly in fp8e4.
  - QKV projection, PV, and the out-projection run as fp8e4 DoubleRow
    matmuls (256-deep contraction at 0.5 cycles/row); scores K^T.T @ Q^T stay
    bf16 (K=64, tile_position row packing) for precision.
  - softmax exp runs on the scalar engine out of PSUM, emitting fp8 with
    exp(0.125*s - 3.2); the -3.2 keeps e4m3 in range and cancels in the
    softmax.  The scalar engine is the bottleneck (~135us of exp), so all
    other scalar-engine work happens during the LN/projection prologue.
  - QK projection + its PSUM->SBUF copies interleave per head pair with the
    attention loop (sharing the scores PSUM ring) so the first exp starts
    ~17us in; denominators accumulate via one-hot fp8 stationaries into one
    [16, S] PSUM tile; their reciprocals broadcast across partitions via a
    tiny selector matmul on the PE (no DRAM round trip) for the ctx
    normalize into fp8.
"""

import numpy as np
import ml_dtypes

P = 128
D = 1024
H = 16
DH = 64
E = 3 * D
B = 8
S = 1024
LN_EPS = 1e-5
N_CORES = 8

_ND = D // P   # d tiles (8)
_NC = 512      # LN chunk
NQ = _ND // 2  # DoubleRow pair steps over the d contraction (4)

LAST_RESULTS = None
_NC_CACHE = {}


def _emit(tc, aps, S_=S):
    from concourse import mybir

    nc = tc.nc
    f32 = mybir.dt.float32
    f32r = mybir.dt.float32r
    bf16 = mybir.dt.bfloat16
    fp8 = mybir.dt.float8e4
    FT = mybir.ActivationFunctionType
    OP = mybir.AluOpType
    DR = mybir.MatmulPerfMode.DoubleRow

    ns = S_ // P                 # s tiles (8)
    ncs = max(1, S_ // _NC)      # LN chunks (2)
    NCK = min(_NC, S_)
    nhp = H // 2                 # head pairs (8)

    xT, xnat, wint, woutT, binqk, out = (
        aps["xt"], aps["xnat"], aps["wint"], aps["woutt"], aps["binqk"],
        aps["out"],
    )
    winT_r = wint.rearrange("(a p) e -> p a e", p=P)
    woutT_r = woutT.rearrange("(a p) e -> p a e", p=P)
    xT_r = xT.rearrange("(a p) s -> p a s", p=P)

    with tc.tile_pool(name="consts", bufs=1) as consts, \
         tc.tile_pool(name="acts", bufs=1) as acts, \
         tc.tile_pool(name="wpool", bufs=1) as wpool:

        # ---------- constants ----------
        ones_mat = consts.tile([P, P], bf16)
        nc.vector.memset(ones_mat, 1.0)
        eps_sb = consts.tile([P, 1], f32)
        nc.vector.memset(eps_sb, LN_EPS)
        # exp input shift: scores (std ~1.39) reach +-8.4; exp(8.4-3.2)=181
        # stays inside e4m3 range, and the shift cancels in the softmax
        expbias = consts.tile([P, 1], f32)
        nc.vector.memset(expbias, -3.2)
        binqk_sb = consts.tile([P, 2 * D // P], f32)
        # one-hot selector for the denominator matmuls ([:, :, h, h] = 1) and
        # the partition-broadcast selector for 1/den (head 2hp -> rows 0..63,
        # head 2hp+1 -> rows 64..127): host-prepared (partition-sliced
        # memsets are illegal on hardware)
        onesel = consts.tile([P, 2, H, H], fp8)
        selmat = consts.tile([H, nhp, P], bf16)
        # identity stationary (float32r): folds the residual add into the
        # out-projection PSUM group on the PE.  (The three const DMAs are
        # issued later, behind the startup-critical xT/winqk transfers.)
        ident_sb = consts.tile([P, P], f32r)

        # ---------- weights ----------
        winv_sb = wpool.tile([P, _ND, D], fp8)       # V columns of in_proj
        winqk_sb = wpool.tile([P, _ND, 2 * D], fp8)  # Q,K columns
        wout_sb = wpool.tile([P, _ND, D], fp8)

        # ---------- persistent activations ----------
        zT_sb = acts.tile([P, _ND, S_], fp8)     # LN'd x (gamma folded out)
        qkT_sb = acts.tile([P, H, S_], bf16)     # q (tiles 0..7), k (8..15)
        v_sb = acts.tile([P, ns, H * DH], fp8)   # v natural [t, e']
        ctxN_sb = acts.tile([P, nhp, S_], bf16)  # unnormalized ctx^T
        ctxT_sb = acts.tile([P, nhp, S_], fp8)   # normalized, for out-proj
        rd_sb = acts.tile([H, S_], bf16)         # 1/den per head
        xnat_sb = acts.tile([P, ns, D], f32r)    # residual (+all bias folds)

        # ============ Phase 1: LayerNorm + V projection ============
        with tc.tile_pool(name="lnsb", bufs=1) as lnsb, \
             tc.tile_pool(name="lnrow", bufs=1) as lnrow, \
             tc.tile_pool(name="lntmp", bufs=2) as lntmp, \
             tc.tile_pool(name="lnps", bufs=1, space="PSUM") as lnps:
            xT_sb = lnsb.tile([P, _ND, S_], bf16)
            # per-chunk stats tiles (separate tiles so chunk-0 readers don't
            # wait on chunk-1 writers)
            sx_c = [lnps.tile([P, NCK], f32, tag=f"sx{c}", name=f"sx_{c}")
                    for c in range(ncs)]
            sx2_c = [lnps.tile([P, NCK], f32, tag=f"sx2{c}", name=f"sx2_{c}")
                     for c in range(ncs)]
            # one batched xT DMA per chunk (per-j DMAs pay ~650ns issue each)
            for c in range(ncs):
                sl = slice(c * NCK, (c + 1) * NCK)
                nc.sync.dma_start(out=xT_sb[:, :, sl], in_=xT_r[:, :, sl])
            nc.sync.dma_start(out=binqk_sb, in_=binqk)
            # priority slices of winqk for the first head pair (et 0 and 8)
            nc.sync.dma_start(out=winqk_sb[:, :, 0:P], in_=winT_r[:, :, 0:P])
            nc.sync.dma_start(out=winqk_sb[:, :, D:D + P],
                              in_=winT_r[:, :, D:D + P])
            nc.sync.dma_start(out=winqk_sb[:, :, P:D], in_=winT_r[:, :, P:D])
            nc.sync.dma_start(out=winqk_sb[:, :, D + P:2 * D],
                              in_=winT_r[:, :, D + P:2 * D])
            # x^2 + stats matmuls for both chunks up front (DVE, bf16 2x)
            for c in range(ncs):
                sl = slice(c * NCK, (c + 1) * NCK)
                for j in range(_ND):
                    sq = lntmp.tile([P, NCK], bf16, tag="sq", bufs=4)
                    with nc.allow_low_precision(reason="x^2 for LN stats in bf16"):
                        nc.vector.tensor_tensor(out=sq, in0=xT_sb[:, j, sl],
                                                in1=xT_sb[:, j, sl], op=OP.mult)
                    nc.tensor.matmul(sx_c[c], lhsT=ones_mat, rhs=xT_sb[:, j, sl],
                                     start=(j == 0), stop=(j == _ND - 1))
                    nc.tensor.matmul(sx2_c[c], lhsT=ones_mat, rhs=sq,
                                     start=(j == 0), stop=(j == _ND - 1))
                if c == 0:
                    nc.sync.dma_start(out=winv_sb, in_=winT_r[:, :, 2 * D:])
                    nc.sync.dma_start(out=onesel, in_=aps["onesel"])
                    nc.sync.dma_start(out=selmat, in_=aps["selmat"])
                    nc.sync.dma_start(out=ident_sb, in_=aps["ident"])

            for c in range(ncs):
                sl = slice(c * NCK, (c + 1) * NCK)
                # stats chain; mu/var on the scalar engine (idle here)
                mu_bc = lnrow.tile([P, NCK], f32, tag="mu", bufs=2)
                nc.scalar.activation(out=mu_bc, in_=sx_c[c], func=FT.Copy,
                                     scale=1.0 / D)
                var_bc = lnrow.tile([P, NCK], f32, tag="var", bufs=2)
                nc.scalar.activation(out=var_bc, in_=sx2_c[c], func=FT.Copy,
                                     scale=1.0 / D)
                musq = lnrow.tile([P, NCK], f32, tag="musq", bufs=2)
                nc.vector.tensor_tensor(out=musq, in0=mu_bc, in1=mu_bc, op=OP.mult)
                nc.vector.tensor_tensor(out=var_bc, in0=var_bc, in1=musq,
                                        op=OP.subtract)
                std_bc = musq
                nc.scalar.activation(out=std_bc, in_=var_bc, func=FT.Sqrt, bias=eps_sb)
                if c == 0:
                    std0_keep = std_bc
                b_bf = lnrow.tile([P, NCK], bf16, tag="bb", bufs=2)
                with nc.allow_low_precision(reason="LN rstd in bf16"):
                    nc.vector.reciprocal(out=b_bf, in_=std_bc)
                mub_bf = lnrow.tile([P, NCK], bf16, tag="mub", bufs=2)
                with nc.allow_low_precision(reason="LN mu*rstd in bf16"):
                    nc.vector.tensor_tensor(out=mub_bf, in0=mu_bc, in1=b_bf,
                                            op=OP.mult)

                # normalize: op1 on DVE (bf16 2x), op2 split DVE/gpsimd
                for j in range(_ND):
                    t = lntmp.tile([P, NCK], bf16, tag="nrm", bufs=4)
                    with nc.allow_low_precision(reason="LN normalize bf16/fp8"):
                        nc.vector.tensor_tensor(out=t, in0=xT_sb[:, j, sl],
                                                in1=b_bf, op=OP.mult)
                        eng = nc.vector if j % 2 == 0 else nc.gpsimd
                        eng.tensor_tensor(out=zT_sb[:, j, sl], in0=t,
                                          in1=mub_bf, op=OP.subtract)

                if c == ncs - 1:
                    # preload the Exp activation table; tdum joins both
                    # chunks' sqrt outputs so the scheduler cannot place any
                    # Sqrt/Copy after this (which would evict the Exp table)
                    tdum = lnrow.tile([P, 1], f32, tag="tdum", bufs=1)
                    nc.vector.tensor_tensor(out=tdum, in0=std0_keep[:, 0:1],
                                            in1=std_bc[:, 0:1], op=OP.add)
                    dummy = lnrow.tile([P, 1], fp8, tag="dummy", bufs=1)
                    with nc.allow_low_precision(reason="act table preload"):
                        nc.scalar.activation(out=dummy, in_=tdum,
                                             func=FT.Exp, scale=0.125,
                                             bias=expbias)
                    nc.sync.dma_start(out=wout_sb, in_=woutT_r)
                    xnat_r = xnat.rearrange("(a p) d -> p a d", p=P)
                    nc.sync.dma_start(out=xnat_sb[:, 0:4, :], in_=xnat_r[:, 0:4, :])
                    nc.sync.dma_start(out=xnat_sb[:, 4:8, :], in_=xnat_r[:, 4:8, :])

        # ============ Phase 2: per head pair: QK proj, scores, exp, PV ====
        # Software-pipelined: iteration hp emits QK for hp+1 and PV/den for
        # hp-1, interleaved between the 16 score tiles of hp, so the PE never
        # head-of-line blocks on a copy and the scalar engine never starves.
        with tc.tile_pool(name="expp", bufs=2) as expp, \
             tc.tile_pool(name="mps", bufs=1, space="PSUM") as mps:

            den_ps = mps.tile([H, S_], f32, tag="den")
            ex_tiles = {}

            def emit_qk_gen(et, g, act_copy=False):
                # QK projection for one e-tile, one [P,512] half (DoubleRow)
                qs = mps.tile([P, 512], f32, tag="qs", bufs=1,
                              name=f"qs{et}_{g}")
                for nb in range(2):
                    nsl_q = slice(g * 512 + nb * 256, g * 512 + (nb + 1) * 256)
                    nsl_p = slice(nb * 256, (nb + 1) * 256)
                    for q in range(NQ):
                        nc.tensor.matmul(
                            qs[:, nsl_p],
                            lhsT=winqk_sb[:, 2 * q:2 * q + 2, et * P:(et + 1) * P],
                            rhs=zT_sb[:, 2 * q:2 * q + 2, nsl_q],
                            start=(q == 0), stop=(q == NQ - 1),
                            perf_mode=DR)
                dst = qkT_sb[:, et, g * 512:(g + 1) * 512]
                if act_copy:
                    # prologue only: the scalar engine is idle pre-exp
                    nc.scalar.activation(out=dst, in_=qs, func=FT.Identity,
                                         bias=binqk_sb[:, et:et + 1], scale=1.0)
                else:
                    nc.vector.tensor_scalar_add(dst, qs, binqk_sb[:, et:et + 1])

            def emit_v_unit(st, g):
                # V projection for one s-tile, one 512-wide e' half; shares
                # the qs psum slot, psum -> fp8 copy on DVE
                vp = mps.tile([P, 512], f32, tag="qs", bufs=1,
                              name=f"vp{st}_{g}")
                for nb in range(2):
                    nsl_v = slice(g * 512 + nb * 256, g * 512 + (nb + 1) * 256)
                    nsl_p = slice(nb * 256, (nb + 1) * 256)
                    for q in range(NQ):
                        nc.tensor.matmul(
                            vp[:, nsl_p],
                            lhsT=zT_sb[:, 2 * q:2 * q + 2, st * P:(st + 1) * P],
                            rhs=winv_sb[:, 2 * q:2 * q + 2, nsl_v],
                            start=(q == 0), stop=(q == NQ - 1),
                            perf_mode=DR)
                with nc.allow_low_precision(reason="v in fp8"):
                    nc.vector.tensor_copy(
                        out=v_sb[:, st, g * 512:(g + 1) * 512], in_=vp)

            def emit_pv_unit(hp, idx, cc):
                h = 2 * hp + idx
                ex_r = ex_tiles[hp]
                pv = mps.tile([DH, 512], f32, tag="pv", bufs=1,
                              name=f"pv{h}_{cc}")
                for nb in range(2):
                    for q in range(NQ):
                        nc.tensor.matmul(
                            pv[:, nb * 256:(nb + 1) * 256],
                            lhsT=v_sb[:, 2 * q:2 * q + 2, h * DH:(h + 1) * DH],
                            rhs=ex_r[:, 2 * q:2 * q + 2,
                                     idx * S_ + cc * 512 + nb * 256:
                                     idx * S_ + cc * 512 + (nb + 1) * 256],
                            start=(q == 0), stop=(q == NQ - 1),
                            perf_mode=DR)
                nc.vector.tensor_copy(
                    out=ctxN_sb[idx * DH:(idx + 1) * DH, hp,
                                cc * 512:(cc + 1) * 512],
                    in_=pv)

            def emit_den_unit(hp, idx, chpair):
                # denominator rows via one-hot stationary into den_ps
                h = 2 * hp + idx
                ex_r = ex_tiles[hp]
                # each 2KB psum zone (two 256-col regions) carries exactly one
                # accumulation group: start on the bank's first matmul, stop
                # on its very last (hp7/idx1/odd-ch/q3)
                for ch in (2 * chpair, 2 * chpair + 1):
                    for q in range(NQ):
                        nc.tensor.matmul(
                            den_ps[:, ch * 256:(ch + 1) * 256],
                            lhsT=onesel[:, :, h, :],
                            rhs=ex_r[:, 2 * q:2 * q + 2,
                                     idx * S_ + ch * 256:idx * S_ + (ch + 1) * 256],
                            start=(hp == 0 and idx == 0 and q == 0 and ch % 2 == 0),
                            stop=(hp == nhp - 1 and idx == 1 and q == NQ - 1
                                  and ch % 2 == 1),
                            perf_mode=DR)

            def pipeline_units(hp):
                units = []
                if hp == 0:
                    for st in range(6):
                        for g in range(2):
                            units.append(("v", st, g))
                elif hp == 1:
                    for st in range(6, ns):
                        for g in range(2):
                            units.append(("v", st, g))
                if hp + 1 < nhp:
                    for g in range(2):
                        for et in (hp + 1, nhp + hp + 1):
                            units.append(("qk", et, g))
                if hp > 0:
                    for idx in range(2):
                        for cc in range(2):
                            units.append(("pv", hp - 1, idx, cc))
                    for idx in range(2):
                        for cc in range(2):
                            units.append(("den", hp - 1, idx, cc))
                return units

            for g in range(2):
                for et in (0, nhp):
                    emit_qk_gen(et, g, act_copy=True)
            for hp in range(nhp):
                ex = expp.tile([P, 16 * S_], fp8, tag="exp", name=f"ex{hp}")
                ex_tiles[hp] = ex.rearrange("p (t x) -> p t x", x=2 * S_)

                units = pipeline_units(hp)
                # scores (bf16, K=64, row-packed) -> exp (fp8), [P,1024] tiles
                for ti in range(16):
                    tt, idx = ti // 2, ti % 2
                    bb = idx * DH
                    base = tt * 2048 + idx * 1024
                    sc2 = mps.tile([P, 1024], f32, tag="sc", bufs=2,
                                   name=f"sc{hp}_{tt}_{idx}")
                    for cc in range(2):
                        nc.tensor.matmul(
                            sc2[:, cc * 512:(cc + 1) * 512],
                            lhsT=qkT_sb[bb:bb + DH, nhp + hp, tt * P:(tt + 1) * P],
                            rhs=qkT_sb[bb:bb + DH, hp, cc * 512:(cc + 1) * 512],
                            start=True, stop=True, tile_position=(bb, 0))
                    with nc.allow_low_precision(reason="softmax exp in fp8"):
                        nc.scalar.activation(out=ex[:, base:base + 1024],
                                             in_=sc2, func=FT.Exp,
                                             scale=0.125, bias=expbias)
                    if ti < len(units):
                        u = units[ti]
                        if u[0] == "qk":
                            emit_qk_gen(u[1], u[2])
                        elif u[0] == "v":
                            emit_v_unit(u[1], u[2])
                        elif u[0] == "pv":
                            emit_pv_unit(u[1], u[2], u[3])
                        else:
                            emit_den_unit(u[1], u[2], u[3])

            # drain: den first (it gates the reciprocals -> tail norms),
            # then PV for the last head pair
            for idx in range(2):
                for cc in range(2):
                    emit_den_unit(nhp - 1, idx, cc)
            for idx in range(2):
                for cc in range(2):
                    emit_pv_unit(nhp - 1, idx, cc)

            # reciprocals of the denominators (bf16, feeds the bc matmul)
            for cc in range(2):
                with nc.allow_low_precision(reason="softmax 1/den in bf16"):
                    nc.vector.reciprocal(out=rd_sb[:, cc * 512:(cc + 1) * 512],
                                         in_=den_ps[:, cc * 512:(cc + 1) * 512])

        # ============ Phase 3: normalize ctx + out-projection ============
        # norms run in s-halves so the first out-proj tiles start after only
        # 8 half-norms; residual adds split across DVE / scalar+gpsimd
        with tc.tile_pool(name="p5", bufs=2) as p5, \
             tc.tile_pool(name="p5ps", bufs=1, space="PSUM") as p5ps:

            def emit_po(st):
                po = p5ps.tile([P, D], f32, tag="po", bufs=3)
                for nb in range(D // 256):
                    nsl = slice(nb * 256, (nb + 1) * 256)
                    for q in range(NQ):
                        nc.tensor.matmul(
                            po[:, nsl],
                            lhsT=ctxT_sb[:, 2 * q:2 * q + 2, st * P:(st + 1) * P],
                            rhs=wout_sb[:, 2 * q:2 * q + 2, nsl],
                            start=(q == 0), stop=False,
                            perf_mode=DR)
                    # residual folded in on the PE: += I @ xnat (float32r)
                    nc.tensor.matmul(po[:, nsl], lhsT=ident_sb,
                                     rhs=xnat_sb[:, st, nsl],
                                     start=False, stop=True)
                # store via scalar engine (idle in the tail); bf16 out
                ot = p5.tile([P, D], bf16, tag="out", bufs=3)
                with nc.allow_low_precision(reason="output store in bf16"):
                    nc.scalar.copy(out=ot, in_=po)
                nc.sync.dma_start(out=out[st * P:(st + 1) * P, :], in_=ot)

            for half in range(2):
                csl = slice(half * 512, (half + 1) * 512)
                for hp in range(nhp):
                    # partition-broadcast 1/den via selector matmul (PE)
                    rdbc = p5ps.tile([P, 512], f32, tag="rdbc", bufs=2,
                                     name=f"rdbc{hp}_{half}")
                    nc.tensor.matmul(rdbc, lhsT=selmat[:, hp, :],
                                     rhs=rd_sb[:, csl], start=True, stop=True)
                    with nc.allow_low_precision(reason="ctx normalize into fp8"):
                        nc.vector.tensor_tensor(out=ctxT_sb[:, hp, csl],
                                                in0=ctxN_sb[:, hp, csl],
                                                in1=rdbc, op=OP.mult)
                for st in range(half * 4, half * 4 + 4):
                    emit_po(st)


def build_nc(S_=S):
    import concourse.bacc as bacc
    import concourse.tile as tile
    from concourse import mybir

    f32 = mybir.dt.float32
    bf16 = mybir.dt.bfloat16
    fp8 = mybir.dt.float8e4

    f32r = mybir.dt.float32r
    nc = bacc.Bacc("TRN2", target_bir_lowering=False, debug=False)
    aps = {
        "xt": nc.dram_tensor("xt", [D, S_], bf16, kind="ExternalInput").ap(),
        "xnat": nc.dram_tensor("xnat", [S_, D], f32r, kind="ExternalInput").ap(),
        "ident": nc.dram_tensor("ident", [P, P], f32r, kind="ExternalInput").ap(),
        "wint": nc.dram_tensor("wint", [D, E], fp8, kind="ExternalInput").ap(),
        "woutt": nc.dram_tensor("woutt", [D, D], fp8, kind="ExternalInput").ap(),
        "binqk": nc.dram_tensor("binqk", [P, 2 * D // P], f32, kind="ExternalInput").ap(),
        "onesel": nc.dram_tensor("onesel", [P, 2, H, H], fp8, kind="ExternalInput").ap(),
        "selmat": nc.dram_tensor("selmat", [H, 8, P], bf16, kind="ExternalInput").ap(),
        "out": nc.dram_tensor("out", [S_, D], bf16, kind="ExternalOutput").ap(),
    }
    with tile.TileContext(nc) as tc:
        _emit(tc, aps, S_)
    nc.compile()
    return nc


def prep_inputs(x, ln_gamma, ln_beta, in_proj_w, in_proj_b, out_proj_w, out_proj_b,
                S_=S, n_cores=N_CORES):
    bf = ml_dtypes.bfloat16
    e4 = ml_dtypes.float8_e4m3
    f32c = lambda a: np.ascontiguousarray(a, dtype=np.float32)

    g = np.asarray(ln_gamma, np.float32)
    bt = np.asarray(ln_beta, np.float32)
    wi = np.asarray(in_proj_w, np.float32)    # [3D, D]
    bi = np.asarray(in_proj_b, np.float32)    # [3D]
    wo = np.asarray(out_proj_w, np.float32)   # [D, D]
    bo = np.asarray(out_proj_b, np.float32)   # [D]

    # fold LN gamma/beta into the in-projection; the V bias passes through
    # softmax additively (sum p = 1), so it folds into the residual via wo.
    wi_f = wi * g[None, :]
    bi_f = bi + wi @ bt
    res_bias = bo + wo @ bi_f[2 * D:]

    onesel = np.zeros((P, 2, H, H), np.float32)
    for h in range(H):
        onesel[:, :, h, h] = 1.0
    selmat = np.zeros((H, H // 2, P), np.float32)
    for hp in range(H // 2):
        selmat[2 * hp, hp, 0:DH] = 1.0
        selmat[2 * hp + 1, hp, DH:P] = 1.0

    shared = {
        "wint": np.ascontiguousarray(wi_f.T).astype(e4),
        "woutt": np.ascontiguousarray(wo.T).astype(e4),
        "binqk": f32c(bi_f[:2 * D].reshape(2 * D // P, P).T),
        "onesel": onesel.astype(e4),
        "selmat": selmat.astype(bf),
        "ident": np.eye(P, dtype=np.float32),
    }
    in_maps = []
    for i in range(n_cores):
        xi = np.asarray(x[i], np.float32)[:S_]
        m = dict(shared)
        m["xt"] = np.ascontiguousarray(xi.T).astype(bf)
        m["xnat"] = f32c(xi + res_bias[None, :])
        in_maps.append(m)
    return in_maps


def kernel(x, ln_gamma, ln_beta, in_proj_w, in_proj_b, out_proj_w, out_proj_b):
    global LAST_RESULTS
    from concourse import bass_utils

    if "nc" not in _NC_CACHE:
        _NC_CACHE["nc"] = build_nc(S)
    nc = _NC_CACHE["nc"]

    in_maps = prep_inputs(x, ln_gamma, ln_beta, in_proj_w, in_proj_b,
                          out_proj_w, out_proj_b)
    res = bass_utils.run_bass_kernel_spmd(nc, in_maps, core_ids=list(range(N_CORES)))
    LAST_RESULTS = res
    out = np.stack([r["out"] for r in res.results], axis=0)
    return np.ascontiguousarray(out, dtype=np.float32)
